# revision 11
# baseline (speedup 1.0000x reference)
"""Trainium2 Bass kernel for the Koopman MLP extractor problem.

Computes, for x [B, 512]:
  sigma = sigmoid(relu(x @ Wc1.T + bc1) @ Wc2.T + bc2)          [B, 2]
  actor = sigma[:,0:1] * (x @ W1.T) + sigma[:,1:2] * (x @ W2.T)  [B, 256]
  q_i   = sum((x @ A_i) * x, 1) + x @ b_i + c_i
  critic = sigma[:,0:1] * q1 + sigma[:,1:2] * q2                 [B, 1]

Strategy: pure data parallel over 8 NeuronCores (8192 rows each). All the
matrix products against x share the contraction dim d=512, so they are fused
into a single wide matmul x @ M with
  M = [A1 | A2 | W1.T | W2.T | Wc1.T | b1 | b2]   [512, 1602]
evaluated per 128-row tile of x as 4 K-chunk accumulating PE matmuls into 4
PSUM column groups. x tiles are transposed on the PE (via identity matmul) to
get the contraction dim onto partitions. Matmuls run in float32r (~1.5e-4 rel
err, 4x the fp32 rate); the epilogue (quadratic row-dots, chooser MLP, gating)
runs in fp32 on the Vector/Scalar/GpSimd engines.
"""

import numpy as np

BATCH = 65536
D = 512
OUT = 256
HID = 64
N_CORES = 8
P = 128
B_LOCAL = BATCH // N_CORES          # 8192 rows per core
N_TILES = B_LOCAL // P              # 64 tiles of 128 rows
GROUP = 16                          # critic assembly batch (tiles)

# column layout of M
NCOL_A1 = D
NCOL_A2 = D
COL_A1 = 0
COL_A2 = COL_A1 + NCOL_A1           # 512
COL_W1 = COL_A2 + NCOL_A2           # 1024
COL_W2 = COL_W1 + OUT               # 1280
COL_WC1 = COL_W2 + OUT              # 1536
COL_B1 = COL_WC1 + HID              # 1600
COL_B2 = COL_B1 + 1                 # 1601
NCOL = COL_B2 + 1                   # 1602
NCOL_G3 = NCOL - COL_WC1            # 66

_CACHE = {}


def _build_program(use_f32r, bc1_nonzero, bc2, c1, c2, n_tiles=N_TILES):
    """Build the per-core Bass program. bc2/c1/c2 are python floats baked in
    as immediates (they are scalars in the model)."""
    import concourse.bass as bass
    import concourse.bacc as bacc
    import concourse.tile as tile
    from concourse import mybir
    from concourse.masks import make_identity

    f32 = mybir.dt.float32
    f32r = mybir.dt.float32r if use_f32r else mybir.dt.float32
    AF = mybir.ActivationFunctionType
    ALU = mybir.AluOpType

    nc = bacc.Bacc("TRN2", target_bir_lowering=False, debug=False)

    b_local = n_tiles * P
    group = min(GROUP, n_tiles)

    x_d = nc.dram_tensor("x", [b_local, D], f32, kind="ExternalInput")
    m_d = nc.dram_tensor("m", [D, NCOL], f32, kind="ExternalInput")
    wc2b_d = nc.dram_tensor("wc2b", [P, 2, HID], f32, kind="ExternalInput")
    bc1e_d = nc.dram_tensor("bc1e", [1, NCOL_G3], f32, kind="ExternalInput")
    actor_d = nc.dram_tensor("actor", [b_local, OUT], f32, kind="ExternalOutput")
    critic_d = nc.dram_tensor("critic", [n_tiles, P], f32, kind="ExternalOutput")

    KC = D // P  # 4 K-chunks

    with tile.TileContext(nc) as tc:
        with tc.tile_pool(name="const", bufs=1) as const, \
             tc.tile_pool(name="io", bufs=3) as io, \
             tc.tile_pool(name="work", bufs=2) as work, \
             tc.tile_pool(name="stage", bufs=1) as stage, \
             tc.tile_pool(name="ps_t", bufs=1, space="PSUM") as ps_t, \
             tc.tile_pool(name="ps_g012", bufs=2, space="PSUM") as ps_g012, \
             tc.tile_pool(name="ps_g3", bufs=1, space="PSUM") as ps_g3:

            ident = const.tile([P, P], f32)
            make_identity(nc, ident[:])

            # --- load + round weights once ---
            m_stage = const.tile([P, KC, NCOL], f32)
            nc.sync.dma_start(
                m_stage[:], m_d.ap().rearrange("(c p) n -> p c n", c=KC, p=P)
            )
            m_r = const.tile([P, KC, NCOL], f32r)
            nc.vector.tensor_copy(
                m_r[:].rearrange("p c n -> p (c n)"),
                m_stage[:].rearrange("p c n -> p (c n)"),
            )
            wc2b = const.tile([P, 2, HID], f32)
            nc.sync.dma_start(wc2b[:], wc2b_d.ap()[:])

            if bc1_nonzero:
                ones_col = const.tile([1, P], f32r)
                nc.gpsimd.memset(ones_col[:], 1.0)
                bc1e_st = const.tile([1, NCOL_G3], f32)
                nc.sync.dma_start(bc1e_st[:], bc1e_d.ap()[:])
                bc1e = const.tile([1, NCOL_G3], f32r)
                nc.vector.tensor_copy(bc1e[:], bc1e_st[:])

            # whole-run staging for critic pieces
            q1_all = stage.tile([P, n_tiles], f32)
            q2_all = stage.tile([P, n_tiles], f32)
            xb_all = stage.tile([P, n_tiles, 2], f32)
            critic_all = stage.tile([P, n_tiles], f32)

            sig_stage = None

            for t in range(n_tiles):
                g = t % group
                if g == 0:
                    sig_stage = work.tile([P, group, 2], f32, tag="sig_stage")

                # load x tile [128 rows, 512]
                x_sb = io.tile([P, D], f32, tag="x_sb")
                nc.sync.dma_start(x_sb[:], x_d.ap()[bass.ts(t, P), :])

                # transpose -> xT (d on partitions), rounded to matmul dtype
                xT_ps = ps_t.tile([P, D], f32, tag="xT_ps")
                for c in range(KC):
                    nc.tensor.transpose(
                        xT_ps[:, bass.ts(c, P)], x_sb[:, bass.ts(c, P)], ident[:]
                    )
                xT = work.tile([P, KC, P], f32r, tag="xT")
                nc.scalar.copy(
                    xT[:].rearrange("p c r -> p (c r)"), xT_ps[:]
                )

                # main matmuls: 4 column groups, accumulate over 4 K-chunks
                g0 = ps_g012.tile([P, 512], f32, tag="g0")  # x@A1
                g1 = ps_g012.tile([P, 512], f32, tag="g1")  # x@A2
                g2 = ps_g012.tile([P, 512], f32, tag="g2")  # y1 | y2
                g3 = ps_g3.tile([P, NCOL_G3], f32, tag="g3")  # h_pre | xb1 | xb2
                groups = [
                    (g0, COL_A1, 512),
                    (g1, COL_A2, 512),
                    (g2, COL_W1, 512),
                    (g3, COL_WC1, NCOL_G3),
                ]
                for c in range(KC):
                    for gt, col0, ncol in groups:
                        stop = c == KC - 1 and not (bc1_nonzero and gt is g3)
                        nc.tensor.matmul(
                            gt[:],
                            xT[:, c, :],
                            m_r[:, c, bass.ds(col0, ncol)],
                            start=(c == 0),
                            stop=stop,
                        )
                if bc1_nonzero:
                    nc.tensor.matmul(
                        g3[:], ones_col[:], bc1e[:], start=False, stop=True
                    )

                # ---- chooser ----
                h = work.tile([P, HID], f32, tag="h")
                nc.scalar.activation(h[:], g3[:, 0:HID], AF.Relu)
                logits = work.tile([P, 2], f32, tag="logits")
                jnk = work.tile([P, HID], f32, tag="jnk")
                for j in range(2):
                    nc.vector.scalar_tensor_tensor(
                        jnk[:],
                        h[:],
                        1.0,
                        wc2b[:, j, :],
                        ALU.mult,
                        ALU.mult,
                        accum_out=logits[:, j : j + 1],
                    )
                sig = sig_stage[:, g, :]
                if bc2 == (0.0, 0.0):
                    nc.scalar.activation(sig, logits[:], AF.Sigmoid)
                else:
                    for j in range(2):
                        nc.scalar.activation(
                            sig[:, j : j + 1],
                            logits[:, j : j + 1],
                            AF.Sigmoid,
                            bias=float(bc2[j]),
                        )
                sig0 = sig_stage[:, g, 0:1]
                sig1 = sig_stage[:, g, 1:2]

                # ---- quadratics: q_i = sum((x@A_i) * x)  (x@b_i staged) ----
                qjnk = work.tile([P, D], f32, tag="qjnk")
                nc.vector.scalar_tensor_tensor(
                    qjnk[:], g0[:], 1.0, x_sb[:], ALU.mult, ALU.mult,
                    accum_out=q1_all[:, t : t + 1],
                )
                nc.vector.scalar_tensor_tensor(
                    qjnk[:], g1[:], 1.0, x_sb[:], ALU.mult, ALU.mult,
                    accum_out=q2_all[:, t : t + 1],
                )
                nc.scalar.copy(xb_all[:, t, :], g3[:, HID : HID + 2])

                # ---- actor = sig0*y1 + sig1*y2 ----
                t1 = work.tile([P, OUT], f32, tag="t1")
                nc.scalar.activation(t1[:], g2[:, 0:OUT], AF.Copy, scale=sig0)
                t2 = work.tile([P, OUT], f32, tag="t2")
                nc.scalar.activation(t2[:], g2[:, OUT:512], AF.Copy, scale=sig1)
                actor_sb = io.tile([P, OUT], f32, tag="actor_sb")
                nc.gpsimd.tensor_add(actor_sb[:], t1[:], t2[:])
                nc.sync.dma_start(actor_d.ap()[bass.ts(t, P), :], actor_sb[:])

                # ---- critic assembly once per GROUP tiles ----
                if g == group - 1:
                    t0 = t - (group - 1)
                    gs = bass.ds(t0, group)
                    q1c = work.tile([P, group], f32, tag="q1c")
                    nc.vector.tensor_add(q1c[:], q1_all[:, gs], xb_all[:, gs, 0])
                    q2c = work.tile([P, group], f32, tag="q2c")
                    nc.vector.tensor_add(q2c[:], q2_all[:, gs], xb_all[:, gs, 1])
                    c_t1 = work.tile([P, group], f32, tag="c_t1")
                    nc.vector.scalar_tensor_tensor(
                        c_t1[:], q1c[:], float(c1),
                        sig_stage[:, :, 0], ALU.add, ALU.mult,
                    )
                    c_t2 = work.tile([P, group], f32, tag="c_t2")
                    nc.vector.scalar_tensor_tensor(
                        c_t2[:], q2c[:], float(c2),
                        sig_stage[:, :, 1], ALU.add, ALU.mult,
                    )
                    nc.vector.tensor_add(critic_all[:, gs], c_t1[:], c_t2[:])

            # transpose critic [128, 64] -> [64, 128] and store
            crit_ps = ps_t.tile([n_tiles, P], f32, tag="xT_ps")
            nc.tensor.transpose(crit_ps[:], critic_all[:], ident[:])
            crit_sb = io.tile([n_tiles, P], f32, tag="crit_sb")
            nc.scalar.copy(crit_sb[:], crit_ps[:])
            nc.sync.dma_start(critic_d.ap()[:], crit_sb[:])

    nc.compile()
    return nc


def _get_program(use_f32r, bc1_nonzero, bc2, c1, c2):
    key = (use_f32r, bc1_nonzero, bc2, c1, c2)
    if key not in _CACHE:
        _CACHE[key] = _build_program(use_f32r, bc1_nonzero, bc2, c1, c2)
    return _CACHE[key]


def kernel(x, W1, W2, A1, b1, c1, A2, b2, c2, Wc1, bc1, Wc2, bc2,
           use_f32r=True, _return_results=False):
    from concourse.bass_utils import run_bass_kernel_spmd

    x = np.asarray(x, dtype=np.float32)
    f = lambda a: np.asarray(a, dtype=np.float32)
    W1, W2, A1, A2, Wc1, Wc2 = map(f, (W1, W2, A1, A2, Wc1, Wc2))
    b1, b2, bc1, bc2, c1, c2 = map(f, (b1, b2, bc1, bc2, c1, c2))

    m = np.concatenate(
        [A1, A2, W1.T, W2.T, Wc1.T, b1[:, None], b2[:, None]], axis=1
    )
    assert m.shape == (D, NCOL)
    m = np.ascontiguousarray(m)
    wc2b = np.ascontiguousarray(
        np.broadcast_to(Wc2[None, :, :], (P, 2, HID))
    )
    bc1_nonzero = bool(np.any(bc1 != 0.0))
    bc1e = np.zeros((1, NCOL_G3), dtype=np.float32)
    bc1e[0, :HID] = bc1

    nc = _get_program(
        use_f32r, bc1_nonzero,
        (float(bc2[0]), float(bc2[1])), float(c1[0]), float(c2[0]),
    )

    in_maps = []
    for core in range(N_CORES):
        xs = x[core * B_LOCAL : (core + 1) * B_LOCAL]
        in_maps.append({"x": xs, "m": m, "wc2b": wc2b, "bc1e": bc1e})

    res = run_bass_kernel_spmd(nc, in_maps, list(range(N_CORES)))

    actor = np.concatenate([r["actor"] for r in res.results], axis=0)
    critic = np.concatenate(
        [r["critic"].reshape(B_LOCAL) for r in res.results], axis=0
    )[:, None]
    if _return_results:
        return actor, critic, res
    return actor, critic
